# revision 9
# baseline (speedup 1.0000x reference)
"""Per-pixel dynamic-filter 5x5 convolution (KPN-style) on 8 TRN2 NeuronCores.

Math: out[b,h,w] = sum_{di,dj,c} img[b, h+di-2, w+dj-2, c] * filts[b, h, w, (di*5+dj)*3+c]
Shapes: img [4,512,512,3] f32, filts [4,512,512,75] f32 -> out [4,512,512] f32.

Strategy (pure data parallel, no cross-core comms):
  - 8 shards = (batch b) x (H half); each core owns a [256, 512] output slab,
    processed as 2 h-tiles of 128 rows (ht outer loop, output overlapped).
  - filts are int8-quantized on host (scale S, ~1e-2 rel err, budget 2e-2) to
    halve the dominant DMA stream; dequantized on-chip to fp16 by the Scalar
    (ACT) and GPSIMD engines in parallel, feeding fp16 2x-mode DVE multiplies.
  - img fp16, one DMA per h-tile bringing all 5 di row-shifted copies
    ([p][di][c][x] with replicated source rows) so engines never need
    partition shifts and the DMA count stays tiny.
  - odd dj operands are 4B-aligned by host-shifting those filts by +1 in w
    (products land in psum cols [1:512); the single real missing w=0 term,
    dj=3, is restored via a tensor_tensor_reduce chain + a free-size-1 matmul).
  - TensorE accumulates all 15 product planes per (ht,di) into one fp32 PSUM
    bank per ht via identity matmuls; ACT evicts to fp16, DMA out.
  - Dummy-matmul warmup in the first DMA shadow lifts the PE HAM throttle.
"""

import sys

sys.path.insert(0, "/opt/trn_rl_repo")

import numpy as np

from concourse import bass, bacc, mybir
from concourse.tile import TileContext
from concourse.bass_utils import run_bass_kernel_spmd

B, H, W, C = 4, 512, 512, 3
K = 5
N_CORES = 8
HSH = H // 2  # 256 rows per shard
N_HT = HSH // 128  # 2 h-tiles per shard
XP = W + 6  # padded x extent: x = w+2, w in [-2, 516)
NPL = K * C  # 15 planes per (ht, di)
N_EVEN = 9  # dj in {0,2,4} x c
N_ODD = 6  # dj in {1,3} x c
SPLIT = 8  # dequant split: ACT does planes [0:SPLIT], GPSIMD [SPLIT:15]
QSCALE = 4.5 / 127.0  # int8 quant scale (clip at 4.5 sigma)
N_WARMUP_MM = 12

_F16 = mybir.dt.float16
_F32 = mybir.dt.float32
_I8 = mybir.dt.int8

_NC = None


def build_nc():
    """Build the single-core Bass program (identical on all 8 cores)."""
    nc = bacc.Bacc("TRN2")
    img_d = nc.declare_dram_parameter("img", [HSH + 4, C, XP], _F16, isOutput=False)
    filts_d = nc.declare_dram_parameter(
        "filts", [128, N_HT, K, NPL, W], _I8, isOutput=False
    )
    edge_d = nc.declare_dram_parameter("edge", [128, N_HT, K, C], _F16, isOutput=False)
    ident_d = nc.declare_dram_parameter("ident", [128, 128], _F16, isOutput=False)
    out_d = nc.declare_dram_parameter("out", [N_HT, 128, W], _F16, isOutput=True)

    with TileContext(nc) as tc:
        with (
            tc.tile_pool(name="const", bufs=1) as constp,
            tc.tile_pool(name="imgp", bufs=2) as imgp,
            tc.tile_pool(name="qp", bufs=3) as qp,
            tc.tile_pool(name="fp", bufs=3) as fp,
            tc.tile_pool(name="prodp", bufs=3) as prodp,
            tc.tile_pool(name="accp", bufs=2) as accp,
            tc.tile_pool(name="outp", bufs=2) as outp,
            tc.tile_pool(name="psump", bufs=2, space="PSUM") as psump,
            tc.tile_pool(name="wpsump", bufs=1, space="PSUM") as wpsump,
        ):
            id_t = constp.tile([128, 128], _F16)
            nc.sync.dma_start(out=id_t[:], in_=ident_d[:])
            edge_t = constp.tile([128, N_HT, K, C], _F16, tag="edge")
            nc.sync.dma_start(out=edge_t[:], in_=edge_d[:])

            # PE warmup: dummy matmuls in the first DMAs' shadow lift HAM.
            wsrc = constp.tile([128, 512], _F16, tag="wsrc")
            nc.gpsimd.memset(wsrc[:], 0.0)
            wps = wpsump.tile([128, 512], _F32)
            for _ in range(N_WARMUP_MM):
                nc.tensor.matmul(wps[:], wsrc[:, :128], wsrc[:], start=True, stop=True)

            for ht in range(N_HT):
                # One DMA for all 5 di row-shifted img copies of this h-tile:
                # tile[p, di, c, x] = img_d[ht*128 + p + di, c, x]
                img_t = imgp.tile([128, K, C, XP], _F16, tag="img", name=f"img{ht}")
                nc.sync.dma_start(
                    out=img_t[:, 0], in_=img_d[ht * 128 : ht * 128 + 128, :, :]
                )
                src = img_d[ht * 128 + 1 : ht * 128 + 129, :, :]
                src = src.unsqueeze(1).broadcast_to((128, K - 1, C, XP)).copy()
                src.ap[1] = [C * XP, K - 1]  # di steps one whole row
                nc.sync.dma_start(out=img_t[:, 1:], in_=src)

                ps = psump.tile([128, W], _F32, tag="ps", name=f"ps{ht}")

                for di in range(K):
                    q_t = qp.tile([128, NPL, W], _I8, tag="q", name=f"q{ht}{di}")
                    nc.sync.dma_start(out=q_t[:], in_=filts_d[:, ht, di])

                    f_t = fp.tile([128, NPL, W], _F16, tag="f", name=f"f{ht}{di}")
                    nc.scalar.mul(f_t[:, :SPLIT], q_t[:, :SPLIT], QSCALE)
                    nc.gpsimd.tensor_scalar_mul(
                        f_t[:, SPLIT:], q_t[:, SPLIT:], QSCALE
                    )

                    # products, even dj in {0,2,4}: img x-offsets {0,2,4}
                    p_e = prodp.tile([128, N_EVEN, W], _F16, tag="pe", name=f"pe{ht}{di}")
                    src_e = (
                        img_t[:, di]
                        .unsqueeze(1)
                        .broadcast_to((128, 3, C, XP))
                        .copy()
                    )
                    src_e.ap[1] = [2, 3]  # dj axis: x offsets 0,2,4
                    src_e.ap[3] = [1, W]
                    nc.vector.tensor_tensor(
                        p_e[:].rearrange("p (a c) x -> p a c x", a=3),
                        src_e,
                        f_t[:, :N_EVEN].rearrange("p (a c) x -> p a c x", a=3),
                        mybir.AluOpType.mult,
                    )

                    # products, odd dj in {1,3}: host-shifted filts (F'[v]=F[v+1],
                    # psum target w=v+1) so img x-offsets are {dj+1} = {2,4}
                    p_o = prodp.tile([128, N_ODD, W], _F16, tag="po", name=f"po{ht}{di}")
                    src_o = (
                        img_t[:, di, :, 2:]
                        .unsqueeze(1)
                        .broadcast_to((128, 2, C, XP - 2))
                        .copy()
                    )
                    src_o.ap[1] = [2, 2]  # dj axis: x offsets 2,4
                    src_o.ap[3] = [1, W]
                    nc.vector.tensor_tensor(
                        p_o[:].rearrange("p (a c) x -> p a c x", a=2),
                        src_o,
                        f_t[:, N_EVEN:].rearrange("p (a c) x -> p a c x", a=2),
                        mybir.AluOpType.mult,
                    )

                    # accumulate the 15 planes into this ht's psum bank
                    for k in range(N_EVEN):
                        nc.tensor.matmul(
                            ps[:],
                            id_t[:],
                            p_e[:, k, :],
                            start=(di == 0 and k == 0),
                            stop=False,
                        )
                    for k in range(N_ODD):
                        nc.tensor.matmul(
                            ps[:, 1:W],
                            id_t[:],
                            p_o[:, k, 0 : W - 1],
                            start=False,
                            stop=(di == K - 1 and k == N_ODD - 1),
                        )

                # w=0 edge terms (dj=3 only): sum_{di,c} img[x=3] * edge filts,
                # via one TT over all (di,c) + a tiny add tree (no TTR on hw)
                ep = accp.tile([128, K, C], _F32, tag="ep", name=f"ep{ht}")
                src_ed = img_t[:, :, :, 3]  # [p, di, c] strided view
                nc.vector.tensor_tensor(
                    ep[:], src_ed, edge_t[:, ht], mybir.AluOpType.mult
                )
                ec = accp.tile([128, K], _F32, tag="ec", name=f"ec{ht}")
                nc.vector.tensor_tensor(
                    ec[:], ep[:, :, 0], ep[:, :, 1], mybir.AluOpType.add
                )
                nc.vector.tensor_tensor(
                    ec[:], ec[:], ep[:, :, 2], mybir.AluOpType.add
                )
                e2 = accp.tile([128, 2], _F32, tag="e2", name=f"e2{ht}")
                nc.vector.tensor_tensor(
                    e2[:], ec[:, 0:2], ec[:, 2:4], mybir.AluOpType.add
                )
                e1 = accp.tile([128, 1], _F32, tag="e1", name=f"e1{ht}")
                nc.vector.tensor_tensor(
                    e1[:], e2[:, 0:1], e2[:, 1:2], mybir.AluOpType.add
                )
                nc.vector.tensor_tensor(
                    e1[:], e1[:], ec[:, 4:5], mybir.AluOpType.add
                )

                # evict: ACT copies cols [1:512); DVE merges edge into col 0
                o_t = outp.tile([128, W], _F16, tag="ot", name=f"ot{ht}")
                nc.scalar.copy(out=o_t[:, 1:], in_=ps[:, 1:])
                nc.vector.tensor_tensor(
                    o_t[:, 0:1], ps[:, 0:1], e1[:], mybir.AluOpType.add
                )
                nc.sync.dma_start(out=out_d[ht], in_=o_t[:])

    nc.compile()
    return nc


def get_nc():
    global _NC
    if _NC is None:
        _NC = build_nc()
    return _NC


def prepare_in_maps(img_stack: np.ndarray, filts: np.ndarray):
    """Shard + reformat FULL fp32 inputs into per-core input maps."""
    ident = np.eye(128, dtype=np.float16)
    in_maps = []
    for core in range(N_CORES):
        b, hh = divmod(core, 2)
        h0 = hh * HSH
        # img: pad h by 2 each side, w by 2 left / 4 right -> [516, 518, 3]
        padded = np.pad(img_stack[b], ((2, 2), (2, XP - W - 2), (0, 0)))
        shard = padded[h0 : h0 + HSH + 4]  # rows h0-2 .. h0+258
        img_p = np.ascontiguousarray(shard.transpose(0, 2, 1)).astype(np.float16)

        # filts -> int8 [p, ht, di, (dj-reordered, c), w] with odd-dj w-shift
        f = filts[b, h0 : h0 + HSH].reshape(N_HT, 128, W, K, K, C)
        q = np.clip(np.round(f / QSCALE), -127, 127).astype(np.int8)
        q = q.transpose(1, 0, 3, 4, 5, 2)  # [p, ht, di, dj, c, w]
        qr = np.empty((128, N_HT, K, K, C, W), dtype=np.int8)
        qr[:, :, :, :3] = q[:, :, :, 0::2]  # dj 0,2,4
        qr[:, :, :, 3:, :, : W - 1] = q[:, :, :, 1::2, :, 1:]  # dj 1,3 shifted
        qr[:, :, :, 3:, :, W - 1] = 0
        filts_p = np.ascontiguousarray(qr.reshape(128, N_HT, K, NPL, W))

        # exact fp16 edge filts: w=0, dj=3 -> [p, ht, di, c]
        edge_p = np.ascontiguousarray(
            f[:, :, 0, :, 3, :].transpose(1, 0, 2, 3)
        ).astype(np.float16)

        in_maps.append(
            {"img": img_p, "filts": filts_p, "edge": edge_p, "ident": ident}
        )
    return in_maps


def assemble_out(results) -> np.ndarray:
    out = np.empty((B, H, W), dtype=np.float32)
    for core in range(N_CORES):
        b, hh = divmod(core, 2)
        out[b, hh * HSH : (hh + 1) * HSH, :] = (
            results[core]["out"].reshape(HSH, W).astype(np.float32)
        )
    return out


def kernel(img_stack: np.ndarray, filts: np.ndarray) -> np.ndarray:
    nc = get_nc()
    in_maps = prepare_in_maps(img_stack, filts)
    res = run_bass_kernel_spmd(nc, in_maps, list(range(N_CORES)))
    return assemble_out(res.results)


# revision 12
# speedup vs baseline: 5.5244x; 5.5244x over previous
"""Per-pixel dynamic-filter 5x5 convolution (KPN-style) on 8 TRN2 NeuronCores.

Math: out[b,h,w] = sum_{di,dj,c} img[b, h+di-2, w+dj-2, c] * filts[b, h, w, (di*5+dj)*3+c]
Shapes: img [4,512,512,3] f32, filts [4,512,512,75] f32 -> out [4,512,512] f32.

Strategy (pure data parallel, no cross-core comms):
  - 8 shards = (batch b) x (H half); each core owns a [256, 512] output slab,
    processed as 2 h-tiles of 128 rows (ht outer loop, output overlapped).
  - filts int8-quantized on host (scale S, ~9e-3 rel err vs 2e-2 budget) to
    halve the dominant DMA stream. Per (ht,di) the 15 (dj,c) planes are
    divided across engines to balance the machine:
      dj in {0,2}   (6 planes): ACT dequant -> DVE fp16 2x TT
      dj = 1        (3 planes): ACT dequant -> DVE fp16 2x TT
      dj = 3 c0,c1  (2 planes): DVE scalar_tensor_tensor on raw int8 (fused)
      dj = 3 c2, dj = 4 (4 planes): GPSIMD int8 x fp16 TT (unscaled products;
        scale folded into an S*I stationary at the PE accumulate step)
  - img fp16; one replicated-row DMA per h-tile carries all 5 di row-shifted
    copies so engines never need partition shifts (DMA count stays tiny).
  - odd dj operands are 4B-aligned by host-shifting those filts by +1 in w
    (x-offset dj+1, psum target cols [1:512)); the single real missing w=0
    term (dj=3) is restored via a small TT add tree merged at eviction.
  - TensorE accumulates the planes into one fp32 PSUM bank per ht via
    identity (or S*identity) matmuls; ACT evicts cols [1:512) to fp16, DVE
    merges col 0 with the edge sum; DMA out.
  - Dummy-matmul warmup in the first DMA shadow lifts the PE HAM throttle.
"""

import sys

sys.path.insert(0, "/opt/trn_rl_repo")

import numpy as np

from concourse import bass, bacc, mybir
from concourse.tile import TileContext
from concourse.bass_utils import run_bass_kernel_spmd

B, H, W, C = 4, 512, 512, 3
K = 5
N_CORES = 8
HSH = H // 2  # 256 rows per shard
N_HT = HSH // 128  # 2 h-tiles per shard
XP = W + 6  # padded x extent: x = w+2, w in [-2, 516)
NPL = K * C  # 15 planes per (ht, di)
QSCALE = float(np.float16(4.5 / 127.0))  # int8 quant scale, fp16-exact
N_WARMUP_MM = 12

_F16 = mybir.dt.float16
_F32 = mybir.dt.float32
_I8 = mybir.dt.int8

_NC = None

# plane order in DRAM: [dj0 c0-2][dj2 c0-2][dj1 c0-2][dj3 c0-2][dj4 c0-2]
# index ranges:         0:3        3:6        6:9       9:12      12:15


def _win(img_t, di, ngrp, x0, stride):
    """img operand AP [p][grp:ngrp step stride][c:3][x:512] at x offset x0."""
    base = img_t[:, di, :, x0:] if x0 else img_t[:, di]
    ap = base.unsqueeze(1).broadcast_to((128, ngrp, C, XP - x0)).copy()
    ap.ap[1] = [stride, ngrp]
    ap.ap[3] = [1, W]
    return ap


def build_nc():
    """Build the single-core Bass program (identical on all 8 cores)."""
    nc = bacc.Bacc("TRN2")
    img_d = nc.declare_dram_parameter("img", [HSH + 4, C, XP], _F16, isOutput=False)
    filts_d = nc.declare_dram_parameter(
        "filts", [128, N_HT, K, NPL, W], _I8, isOutput=False
    )
    edge_d = nc.declare_dram_parameter("edge", [128, N_HT, K, C], _F16, isOutput=False)
    ident_d = nc.declare_dram_parameter("ident", [128, 128], _F16, isOutput=False)
    out_d = nc.declare_dram_parameter("out", [N_HT, 128, W], _F16, isOutput=True)

    with TileContext(nc) as tc:
        with (
            tc.tile_pool(name="const", bufs=1) as constp,
            tc.tile_pool(name="imgp", bufs=2) as imgp,
            tc.tile_pool(name="qp", bufs=3) as qp,
            tc.tile_pool(name="fp", bufs=3) as fp,
            tc.tile_pool(name="prodp", bufs=3) as prodp,
            tc.tile_pool(name="accp", bufs=2) as accp,
            tc.tile_pool(name="outp", bufs=2) as outp,
            tc.tile_pool(name="psump", bufs=2, space="PSUM") as psump,
            tc.tile_pool(name="wpsump", bufs=1, space="PSUM") as wpsump,
        ):
            id_t = constp.tile([128, 128], _F16)
            nc.sync.dma_start(out=id_t[:], in_=ident_d[:])
            ids_t = constp.tile([128, 128], _F16, tag="ids")
            nc.scalar.mul(ids_t[:], id_t[:], QSCALE)  # S*I for unscaled planes
            edge_t = constp.tile([128, N_HT, K, C], _F16, tag="edge")
            nc.sync.dma_start(out=edge_t[:], in_=edge_d[:])

            # PE warmup: dummy matmuls in the first DMAs' shadow lift HAM.
            wsrc = constp.tile([128, 512], _F16, tag="wsrc")
            nc.vector.memset(wsrc[:], 0.0)
            wps = wpsump.tile([128, 512], _F32)
            for _ in range(N_WARMUP_MM):
                nc.tensor.matmul(wps[:], wsrc[:, :128], wsrc[:], start=True, stop=True)

            for ht in range(N_HT):
                # One DMA (split in two for startup) for all 5 di row-shifted
                # img copies: tile[p, di, c, x] = img_d[ht*128 + p + di, c, x]
                img_t = imgp.tile([128, K, C, XP], _F16, tag="img", name=f"img{ht}")
                nc.sync.dma_start(
                    out=img_t[:, 0], in_=img_d[ht * 128 : ht * 128 + 128, :, :]
                )
                src = img_d[ht * 128 + 1 : ht * 128 + 129, :, :]
                src = src.unsqueeze(1).broadcast_to((128, K - 1, C, XP)).copy()
                src.ap[1] = [C * XP, K - 1]  # di steps one whole row
                nc.sync.dma_start(out=img_t[:, 1:], in_=src)

                ps = psump.tile([128, W], _F32, tag="ps", name=f"ps{ht}")

                for di in range(K):
                    q_t = qp.tile([128, NPL, W], _I8, tag="q", name=f"q{ht}{di}")
                    nc.sync.dma_start(out=q_t[:], in_=filts_d[:, ht, di])

                    # ACT dequants planes 0:9 (dj 0,2,1)
                    f_t = fp.tile([128, 9, W], _F16, tag="f", name=f"f{ht}{di}")
                    nc.scalar.mul(f_t[:], q_t[:, :9], QSCALE)

                    # DVE TT-A: dj in {0,2}, x0 {0,2}  (6 planes, fp16 2x)
                    p_a = prodp.tile([128, 6, W], _F16, tag="pa", name=f"pa{ht}{di}")
                    nc.vector.tensor_tensor(
                        p_a[:].rearrange("p (a c) x -> p a c x", a=2),
                        _win(img_t, di, 2, 0, 2),
                        f_t[:, :6].rearrange("p (a c) x -> p a c x", a=2),
                        mybir.AluOpType.mult,
                    )
                    # DVE TT-B: dj=1, x0=2  (3 planes, fp16 2x)
                    p_b = prodp.tile([128, 3, W], _F16, tag="pb", name=f"pb{ht}{di}")
                    nc.vector.tensor_tensor(
                        p_b[:].rearrange("p (a c) x -> p a c x", a=1),
                        _win(img_t, di, 1, 2, 2),
                        f_t[:, 6:9].rearrange("p (a c) x -> p a c x", a=1),
                        mybir.AluOpType.mult,
                    )
                    # DVE STT: dj=3 c0,c1, x0=4 (2 planes, int8 fused dequant)
                    p_s = prodp.tile([128, 2, W], _F16, tag="ps2", name=f"pS{ht}{di}")
                    nc.vector.scalar_tensor_tensor(
                        p_s[:],
                        q_t[:, 9:11],
                        QSCALE,
                        img_t[:, di, 0:2, 4 : 4 + W],
                        mybir.AluOpType.mult,
                        mybir.AluOpType.mult,
                    )
                    # GPSIMD TT: dj=3 c2 (1 plane) + dj=4 (3 planes), int8
                    # unscaled; their matmuls use the S*I stationary.
                    p_g1 = prodp.tile([128, 1, W], _F16, tag="pg1", name=f"pg1{ht}{di}")
                    nc.gpsimd.tensor_tensor(
                        p_g1[:, 0],
                        q_t[:, 11],
                        img_t[:, di, 2, 4 : 4 + W],
                        mybir.AluOpType.mult,
                    )
                    p_g3 = prodp.tile([128, 3, W], _F16, tag="pg3", name=f"pg3{ht}{di}")
                    nc.gpsimd.tensor_tensor(
                        p_g3[:],
                        q_t[:, 12:15],
                        img_t[:, di, :, 4 : 4 + W],
                        mybir.AluOpType.mult,
                    )

                    # accumulate 15 planes into this ht's psum bank.
                    # evens (full 512): p_a (id), p_g3 (S*I)
                    for k in range(6):
                        nc.tensor.matmul(
                            ps[:],
                            id_t[:],
                            p_a[:, k, :],
                            start=(di == 0 and k == 0),
                            stop=False,
                        )
                    for k in range(3):
                        nc.tensor.matmul(
                            ps[:], ids_t[:], p_g3[:, k, :], start=False, stop=False
                        )
                    # odds (cols [1:512)): p_b (id), p_s (id), p_g1 (S*I)
                    for k in range(3):
                        nc.tensor.matmul(
                            ps[:, 1:W],
                            id_t[:],
                            p_b[:, k, 0 : W - 1],
                            start=False,
                            stop=False,
                        )
                    for k in range(2):
                        nc.tensor.matmul(
                            ps[:, 1:W],
                            id_t[:],
                            p_s[:, k, 0 : W - 1],
                            start=False,
                            stop=False,
                        )
                    nc.tensor.matmul(
                        ps[:, 1:W],
                        ids_t[:],
                        p_g1[:, 0, 0 : W - 1],
                        start=False,
                        stop=(di == K - 1),
                    )

                # w=0 edge terms (dj=3 only): sum_{di,c} img[x=3] * edge filts
                ep = accp.tile([128, K, C], _F32, tag="ep", name=f"ep{ht}")
                nc.vector.tensor_tensor(
                    ep[:], img_t[:, :, :, 3], edge_t[:, ht], mybir.AluOpType.mult
                )
                ec = accp.tile([128, K], _F32, tag="ec", name=f"ec{ht}")
                nc.vector.tensor_tensor(
                    ec[:], ep[:, :, 0], ep[:, :, 1], mybir.AluOpType.add
                )
                nc.vector.tensor_tensor(
                    ec[:], ec[:], ep[:, :, 2], mybir.AluOpType.add
                )
                e2 = accp.tile([128, 2], _F32, tag="e2", name=f"e2{ht}")
                nc.vector.tensor_tensor(
                    e2[:], ec[:, 0:2], ec[:, 2:4], mybir.AluOpType.add
                )
                e1 = accp.tile([128, 1], _F32, tag="e1", name=f"e1{ht}")
                nc.vector.tensor_tensor(
                    e1[:], e2[:, 0:1], e2[:, 1:2], mybir.AluOpType.add
                )
                nc.vector.tensor_tensor(
                    e1[:], e1[:], ec[:, 4:5], mybir.AluOpType.add
                )

                # evict: ACT copies cols [1:512); DVE merges edge into col 0
                o_t = outp.tile([128, W], _F16, tag="ot", name=f"ot{ht}")
                nc.scalar.copy(out=o_t[:, 1:], in_=ps[:, 1:])
                nc.vector.tensor_tensor(
                    o_t[:, 0:1], ps[:, 0:1], e1[:], mybir.AluOpType.add
                )
                nc.sync.dma_start(out=out_d[ht], in_=o_t[:])

    nc.compile()
    return nc


def get_nc():
    global _NC
    if _NC is None:
        _NC = build_nc()
    return _NC


def prepare_in_maps(img_stack: np.ndarray, filts: np.ndarray):
    """Shard + reformat FULL fp32 inputs into per-core input maps."""
    ident = np.eye(128, dtype=np.float16)
    in_maps = []
    for core in range(N_CORES):
        b, hh = divmod(core, 2)
        h0 = hh * HSH
        # img: pad h by 2 each side, w by 2 left / 4 right -> [516, 518, 3]
        padded = np.pad(img_stack[b], ((2, 2), (2, XP - W - 2), (0, 0)))
        shard = padded[h0 : h0 + HSH + 4]  # rows h0-2 .. h0+258
        img_p = np.ascontiguousarray(shard.transpose(0, 2, 1)).astype(np.float16)

        # filts -> int8 [p, ht, di, plane, w]; plane order
        # [dj0 c0-2][dj2][dj1][dj3][dj4], odd dj (1,3) shifted +1 in w
        f = filts[b, h0 : h0 + HSH].reshape(N_HT, 128, W, K, K, C)
        q = np.clip(np.round(f / QSCALE), -127, 127).astype(np.int8)
        q = q.transpose(1, 0, 3, 4, 5, 2)  # [p, ht, di, dj, c, w]
        qr = np.empty((128, N_HT, K, K, C, W), dtype=np.int8)
        qr[:, :, :, 0] = q[:, :, :, 0]
        qr[:, :, :, 1] = q[:, :, :, 2]
        qr[:, :, :, 4] = q[:, :, :, 4]
        qr[:, :, :, 2:4, :, : W - 1] = q[:, :, :, 1::2, :, 1:]  # dj 1,3 shifted
        qr[:, :, :, 2:4, :, W - 1] = 0
        filts_p = np.ascontiguousarray(qr.reshape(128, N_HT, K, NPL, W))

        # exact fp16 edge filts: w=0, dj=3 -> [p, ht, di, c]
        edge_p = np.ascontiguousarray(
            f[:, :, 0, :, 3, :].transpose(1, 0, 2, 3)
        ).astype(np.float16)

        in_maps.append(
            {"img": img_p, "filts": filts_p, "edge": edge_p, "ident": ident}
        )
    return in_maps


def assemble_out(results) -> np.ndarray:
    out = np.empty((B, H, W), dtype=np.float32)
    for core in range(N_CORES):
        b, hh = divmod(core, 2)
        out[b, hh * HSH : (hh + 1) * HSH, :] = (
            results[core]["out"].reshape(HSH, W).astype(np.float32)
        )
    return out


def kernel(img_stack: np.ndarray, filts: np.ndarray) -> np.ndarray:
    nc = get_nc()
    in_maps = prepare_in_maps(img_stack, filts)
    res = run_bass_kernel_spmd(nc, in_maps, list(range(N_CORES)))
    return assemble_out(res.results)
